# revision 1
# baseline (speedup 1.0000x reference)
"""Trainium2 Bass kernel for nn_ConductivityPredictor (GNN message passing).

Strategy (8 NeuronCores, SPMD):
  - Shard nodes/graphs across cores by graph id (batch is sorted -> contiguous
    node ranges). Each core owns ~6250 nodes / 32 graphs; dense weights are
    replicated.
  - Activations live in SBUF channel-major (hT: [512 chan, Np nodes], bf16).
  - Per layer:
      m1T = gelu(W1.T @ hT + b1)            (channel-major matmul, ACT-fused bias+gelu)
      msg2 = (m1 @ W2)                      (node-major output; the @W2 is folded
                                             BEFORE the scatter: mean(msg[src]) @ W2
                                             == mean((msg @ W2)[src]))
      AllGather msg2 across cores -> full table in DRAM
      edge gather (dma_gather, dst-sorted edge chunks of 128)
      scatter-mean via one-hot matmuls: aggT[chan,dst] += G_chunk[:,chan].T @ P_chunk
        (P carries 1/deg weights; the channel-major output gives the layout
         transpose needed between chained matmuls for free)
      hT = gelu(aggT + b2)                  (ACT-fused, channel-major)
  - Readout: z = h @ head_W via matmul with M=1, transpose z via a strided DMA,
    per-graph pooling via matmul with a host-built pool matrix, scale+bias on ACT.

All data-dependent structure (chunk counts per dst range, per-core padding) is
computed on the host from the actual edge data and padded to the max over cores
so a single SPMD program works for all 8 cores.
"""

import math

import numpy as np
import ml_dtypes

import concourse.bacc as bacc
import concourse.bass as bass  # noqa: F401  (kept for debugging)
import concourse.mybir as mybir
import concourse.tile as tile
from concourse.bass_utils import run_bass_kernel_spmd
from concourse.tile import add_dep_helper

BF16 = mybir.dt.bfloat16
F32 = mybir.dt.float32
I16 = mybir.dt.int16
P = 128

bf16 = ml_dtypes.bfloat16


class Plan:
    """Uniform (cross-core) structure description."""


def _wrap_idx(ids):
    """int array (len % 16 == 0) -> [128, len/16] int16 tile: 16-partition wrap
    (idx i at [i % 16, i // 16]), replicated 8x down partitions for the 8 Q7
    gpsimd cores."""
    n = len(ids)
    a = np.asarray(ids, dtype=np.int16).reshape(n // 16, 16).T
    return np.tile(a, (8, 1))


def preprocess(inputs, n_cores=8, n_graphs=None):
    x = np.asarray(inputs["x"], dtype=np.float32)
    edge_index = np.asarray(inputs["edge_index"], dtype=np.int64)
    batch = np.asarray(inputs["batch"], dtype=np.int64)
    embed_W = np.asarray(inputs["embed_W"], dtype=np.float32)
    embed_b = np.asarray(inputs["embed_b"], dtype=np.float32)
    W1 = np.asarray(inputs["W1"], dtype=np.float32)
    b1 = np.asarray(inputs["b1"], dtype=np.float32)
    W2 = np.asarray(inputs["W2"], dtype=np.float32)
    b2 = np.asarray(inputs["b2"], dtype=np.float32)
    head_W = np.asarray(inputs["head_W"], dtype=np.float32)
    head_b = np.asarray(inputs["head_b"], dtype=np.float32)

    N, F = x.shape
    C = embed_W.shape[1]
    L = W1.shape[0]
    G = n_graphs if n_graphs is not None else int(batch.max()) + 1
    assert G % n_cores == 0, (G, n_cores)
    gpc = G // n_cores

    src = edge_index[0].astype(np.int64)
    dst = edge_index[1].astype(np.int64)

    cuts = np.searchsorted(batch, np.arange(n_cores + 1) * gpc).astype(np.int64)
    nd = np.diff(cuts)
    NP = int(math.ceil(max(int(nd.max()), 1) / 512) * 512)
    T = NP // P          # 128-node tiles per core
    R = NP // P          # dst ranges of width 128
    SR = NP // 512       # gather super-ranges (4 ranges each)
    NB = NP // 512
    NTOT = n_cores * NP
    SPLIT = (NTOT // 2 + P - 1) // P * P
    assert SPLIT <= 32768 and (NTOT - SPLIT) <= 32768, (NTOT, SPLIT)

    owner = np.searchsorted(cuts, src, side="right") - 1
    src_pid = owner * NP + (src - cuts[owner])

    deg = np.bincount(dst, minlength=N)
    inv_deg = (1.0 / np.maximum(deg, 1)).astype(np.float32)

    # ---- per-core edge grouping (sorted by dst range, then src half) ----
    per_core = []
    counts = np.zeros((n_cores, R, 2), dtype=np.int64)
    for d in range(n_cores):
        m = (dst >= cuts[d]) & (dst < cuts[d + 1])
        e_dst_loc = (dst[m] - cuts[d]).astype(np.int64)
        e_src = src_pid[m]
        e_w = inv_deg[dst[m]]
        r = e_dst_loc // P
        half = (e_src >= SPLIT).astype(np.int64)
        order = np.lexsort((e_src, half, r))
        per_core.append(
            (r[order], half[order], e_src[order], (e_dst_loc % P)[order], e_w[order])
        )
        cnt = np.bincount(r * 2 + half, minlength=R * 2).reshape(R, 2)
        counts[d] = cnt

    # chunk counts per (r, half): max over cores; lo forced >= 1 so every dst
    # range gets its epilogue (agg=0 -> gelu(b2)) even with no edges.
    nchunks = (counts + P - 1) // P
    ncl = nchunks[:, :, 0].max(axis=0)
    nch = nchunks[:, :, 1].max(axis=0)
    ncl = np.maximum(ncl, 1)

    NCL_sr = [int(ncl[4 * s : 4 * s + 4].sum()) for s in range(SR)]
    NCH_sr = [int(nch[4 * s : 4 * s + 4].sum()) for s in range(SR)]
    NCHUNKS = int(ncl.sum() + nch.sum())

    # idx tensor layout: per sr: lo group then hi group (units: cols = idxs/16)
    idx_off = {}
    off = 0
    for s in range(SR):
        idx_off[(s, 0)] = off
        off += NCL_sr[s] * 8
        idx_off[(s, 1)] = off
        off += NCH_sr[s] * 8
    IDXCOLS = max(off, 8)

    # P-matrix chunk layout: per sr: lo chunks (r asc, c asc) then hi chunks
    pm_off = {}
    off = 0
    for s in range(SR):
        pm_off[s] = off
        off += NCL_sr[s] + NCH_sr[s]
    assert off == NCHUNKS

    plan = Plan()
    plan.n_cores = n_cores
    plan.N, plan.F, plan.C, plan.L, plan.G, plan.gpc = N, F, C, L, G, gpc
    plan.NP, plan.T, plan.R, plan.SR = NP, T, R, SR
    plan.NTOT, plan.SPLIT = NTOT, SPLIT
    plan.ncl, plan.nch = ncl, nch
    plan.NCL_sr, plan.NCH_sr = NCL_sr, NCH_sr
    plan.NCHUNKS, plan.IDXCOLS = NCHUNKS, IDXCOLS
    plan.idx_off, plan.pm_off = idx_off, pm_off
    plan.NB = NB
    plan.CK = C // P

    # ---- shared weight tensors ------------------------------------------
    CK = plan.CK
    embW = np.zeros((P, C), dtype=bf16)
    embW[:F, :] = embed_W.astype(bf16)
    embB = np.ascontiguousarray(embed_b.reshape(CK, P).T.astype(np.float32))
    W1ALL = np.ascontiguousarray(
        W1.reshape(L, CK, P, C).transpose(2, 0, 1, 3).reshape(P, L * CK * C)
    ).astype(bf16)
    W2ALL = np.ascontiguousarray(
        W2.reshape(L, CK, P, C).transpose(2, 0, 1, 3).reshape(P, L * CK * C)
    ).astype(bf16)
    B1ALL = np.ascontiguousarray(
        b1.reshape(L, CK, P).transpose(2, 0, 1).reshape(P, L * CK)
    ).astype(np.float32)
    B2ALL = np.ascontiguousarray(
        b2.reshape(L, CK, P).transpose(2, 0, 1).reshape(P, L * CK)
    ).astype(np.float32)
    HWm = np.ascontiguousarray(head_W.reshape(CK, P).T).astype(bf16)

    # ---- per-core tensors ------------------------------------------------
    in_maps = []
    for d in range(n_cores):
        n_loc = int(nd[d])
        xT = np.zeros((P, NP), dtype=bf16)
        xT[:F, :n_loc] = x[cuts[d] : cuts[d + 1]].T.astype(bf16)

        POOLM = np.zeros((P, T * gpc), dtype=bf16)
        bl = (batch[cuts[d] : cuts[d + 1]] - d * gpc).astype(np.int64)
        node_ids = np.arange(n_loc)
        POOLM[node_ids % P, (node_ids // P) * gpc + bl] = 1.0
        cnts = np.bincount(bl, minlength=gpc).astype(np.float32)
        RCm = (1.0 / np.maximum(cnts, 1.0)).reshape(gpc, 1).astype(np.float32)
        HBm = np.full((gpc, 1), float(head_b.reshape(-1)[0]), dtype=np.float32)

        r_arr, half_arr, srcp_arr, dsto_arr, w_arr = per_core[d]
        IDX = np.zeros((P, IDXCOLS), dtype=np.int16)
        PMAT = np.zeros((P, NCHUNKS * P), dtype=bf16)
        for s in range(SR):
            for h in (0, 1):
                ncs = ncl if h == 0 else nch
                ids_parts = []
                for r in range(4 * s, 4 * s + 4):
                    sel = (r_arr == r) & (half_arr == h)
                    e_ids = srcp_arr[sel] - h * SPLIT
                    e_dst = dsto_arr[sel]
                    e_w = w_arr[sel]
                    npad = int(ncs[r]) * P
                    assert len(e_ids) <= npad, (d, s, h, r, len(e_ids), npad)
                    ids = np.zeros(npad, dtype=np.int64)
                    ids[: len(e_ids)] = e_ids
                    ids_parts.append(ids)
                    # chunk position of this r within the sr tile
                    if h == 0:
                        pos = int(ncl[4 * s : r].sum())
                    else:
                        pos = NCL_sr[s] + int(nch[4 * s : r].sum())
                    for c in range(int(ncs[r])):
                        lo_e = c * P
                        hi_e = min((c + 1) * P, len(e_ids))
                        if hi_e <= lo_e:
                            continue  # all-pad chunk -> stays zero
                        Pm = np.zeros((P, P), dtype=np.float32)
                        kk = np.arange(lo_e, hi_e)
                        np.add.at(Pm, (kk - lo_e, e_dst[kk]), e_w[kk])
                        col0 = (pm_off[s] + pos + c) * P
                        PMAT[:, col0 : col0 + P] = Pm.astype(bf16)
                ids_all = np.concatenate(ids_parts) if ids_parts else None
                if ids_all is not None and len(ids_all):
                    col0 = idx_off[(s, h)]
                    w = _wrap_idx(ids_all)
                    IDX[:, col0 : col0 + w.shape[1]] = w

        in_maps.append(
            {
                "xt": xT,
                "idx": IDX,
                "pmat": PMAT,
                "poolm": POOLM,
                "rc": RCm,
                "hb": HBm,
                "embw": embW,
                "embb": embB,
                "w1all": W1ALL,
                "w2all": W2ALL,
                "b1all": B1ALL,
                "b2all": B2ALL,
                "hw": HWm,
            }
        )

    return plan, in_maps, cuts


# ----------------------------------------------------------------------------
# Bass program
# ----------------------------------------------------------------------------


def build_program(plan):
    n_cores = plan.n_cores
    NP, T, SR = plan.NP, plan.T, plan.SR
    NTOT, SPLIT = plan.NTOT, plan.SPLIT
    C, L, CK, NB, gpc = plan.C, plan.L, plan.CK, plan.NB, plan.gpc
    ncl, nch = plan.ncl, plan.nch
    NCL_sr, NCH_sr = plan.NCL_sr, plan.NCH_sr

    nc = bacc.Bacc("TRN2", debug=False, num_devices=n_cores, name="gnn_mp")

    XT = nc.declare_dram_parameter("xt", [P, NP], BF16, isOutput=False)
    IDX = nc.declare_dram_parameter("idx", [P, plan.IDXCOLS], I16, isOutput=False)
    PMAT = nc.declare_dram_parameter("pmat", [P, plan.NCHUNKS * P], BF16, isOutput=False)
    POOLM = nc.declare_dram_parameter("poolm", [P, T * gpc], BF16, isOutput=False)
    RC = nc.declare_dram_parameter("rc", [gpc, 1], F32, isOutput=False)
    HB = nc.declare_dram_parameter("hb", [gpc, 1], F32, isOutput=False)
    EMBW = nc.declare_dram_parameter("embw", [P, C], BF16, isOutput=False)
    EMBB = nc.declare_dram_parameter("embb", [P, CK], F32, isOutput=False)
    W1ALL = nc.declare_dram_parameter("w1all", [P, L * CK * C], BF16, isOutput=False)
    W2ALL = nc.declare_dram_parameter("w2all", [P, L * CK * C], BF16, isOutput=False)
    B1ALL = nc.declare_dram_parameter("b1all", [P, L * CK], F32, isOutput=False)
    B2ALL = nc.declare_dram_parameter("b2all", [P, L * CK], F32, isOutput=False)
    HWP = nc.declare_dram_parameter("hw", [P, CK], BF16, isOutput=False)
    Y = nc.declare_dram_parameter("y", [gpc, 1], F32, isOutput=True)

    msg2_loc = [nc.dram_tensor(f"msg2loc{l}", [NP, C], BF16) for l in range(L)]
    msg2_all = [
        nc.dram_tensor(f"msg2all{l}", [NTOT, C], BF16, addr_space="Shared")
        for l in range(L)
    ]
    ZD = nc.dram_tensor("zdram", [NP], F32)

    max_ncl = max(NCL_sr)
    max_nch = max(max(NCH_sr), 1)
    max_nc_sr = max(NCL_sr[s] + NCH_sr[s] for s in range(SR))

    with tile.TileContext(nc) as tc:
        with (
            tc.tile_pool(name="res", bufs=1) as res,
            tc.tile_pool(name="wpool", bufs=2) as wpool,
            tc.tile_pool(name="m1pool", bufs=2) as m1pool,
            tc.tile_pool(name="mpool", bufs=4) as mpool,
            tc.tile_pool(name="gpool", bufs=2) as gpool,
            tc.tile_pool(name="ppool", bufs=2) as ppool,
            tc.tile_pool(name="pmm", bufs=2, space="PSUM") as pmm,
            tc.tile_pool(name="pm2", bufs=2, space="PSUM") as pm2,
            tc.tile_pool(name="psc", bufs=4, space="PSUM") as psc,
        ):
            # ---------- resident loads ----------
            def load(dram, shape, dtype, name):
                t = res.tile(shape, dtype, name=name, tag=name)
                nc.sync.dma_start(out=t[:], in_=dram[:])
                return t

            xt = load(XT, [P, NP], BF16, "xt_sb")
            idxsb = load(IDX, [P, plan.IDXCOLS], I16, "idx_sb")
            poolm = load(POOLM, [P, T * gpc], BF16, "poolm_sb")
            rcsb = load(RC, [gpc, 1], F32, "rc_sb")
            hbsb = load(HB, [gpc, 1], F32, "hb_sb")
            embw = load(EMBW, [P, C], BF16, "embw_sb")
            embb = load(EMBB, [P, CK], F32, "embb_sb")
            b1sb = load(B1ALL, [P, L * CK], F32, "b1_sb")
            b2sb = load(B2ALL, [P, L * CK], F32, "b2_sb")
            hwsb = load(HWP, [P, CK], BF16, "hw_sb")

            hT = [res.tile([P, NP], BF16, name=f"hT{i}", tag=f"hT{i}") for i in range(CK)]

            # ---------- embed ----------
            for nb in range(NB):
                ns = slice(nb * 512, (nb + 1) * 512)
                for j in range(CK):
                    ps = pmm.tile([P, 512], F32, tag="mm")
                    nc.tensor.matmul(
                        ps[:],
                        lhsT=embw[:, j * P : (j + 1) * P],
                        rhs=xt[:, ns],
                        start=True,
                        stop=True,
                    )
                    nc.scalar.activation(
                        hT[j][:, ns],
                        ps[:],
                        mybir.ActivationFunctionType.Identity,
                        bias=embb[:, j : j + 1],
                    )

            # ---------- layers ----------
            for l in range(L):
                w1 = wpool.tile([P, CK * C], BF16, tag="w1")
                nc.sync.dma_start(out=w1[:], in_=W1ALL[:, l * CK * C : (l + 1) * CK * C])
                w2 = wpool.tile([P, CK * C], BF16, tag="w2")
                nc.sync.dma_start(out=w2[:], in_=W2ALL[:, l * CK * C : (l + 1) * CK * C])

                # --- m1 (channel-major) then m2 (node-major), per node block
                for nb in range(NB):
                    ns = slice(nb * 512, (nb + 1) * 512)
                    m1sb = []
                    for j in range(CK):
                        ps = pmm.tile([P, 512], F32, tag="mm")
                        for i in range(CK):
                            nc.tensor.matmul(
                                ps[:],
                                lhsT=w1[:, i * C + j * P : i * C + j * P + P],
                                rhs=hT[i][:, ns],
                                start=(i == 0),
                                stop=(i == CK - 1),
                            )
                        m1 = m1pool.tile([P, 512], BF16, tag=f"m1_{j}")
                        nc.scalar.activation(
                            m1[:],
                            ps[:],
                            mybir.ActivationFunctionType.Gelu,
                            bias=b1sb[:, l * CK + j : l * CK + j + 1],
                        )
                        m1sb.append(m1)
                    for t in range(4):
                        ps2 = pm2.tile([P, 512], F32, tag="m2")
                        for j in range(CK):
                            nc.tensor.matmul(
                                ps2[:],
                                lhsT=m1sb[j][:, t * P : (t + 1) * P],
                                rhs=w2[:, j * C : (j + 1) * C],
                                start=(j == 0),
                                stop=(j == CK - 1),
                            )
                        ms = mpool.tile([P, 512], BF16, tag="msg")
                        nc.vector.tensor_copy(ms[:], ps2[:])
                        row0 = (nb * 4 + t) * P
                        nc.sync.dma_start(out=msg2_loc[l][row0 : row0 + P, :], in_=ms[:])

                # --- AllGather
                cc = nc.gpsimd.collective_compute(
                    "AllGather",
                    mybir.AluOpType.bypass,
                    replica_groups=[list(range(n_cores))],
                    ins=[msg2_loc[l][:, :]],
                    outs=[msg2_all[l][:, :]],
                )

                # --- gather + scatter-mean + update, per super-range
                lo_tab = msg2_all[l][0:SPLIT, :]
                hi_tab = msg2_all[l][SPLIT:NTOT, :]
                for s in range(SR):
                    nclo, nchi = NCL_sr[s], NCH_sr[s]
                    g0 = gpool.tile([P, max_ncl * C], BF16, tag="g0")
                    ni = nclo * P
                    gi = nc.gpsimd.dma_gather(
                        g0[:, : nclo * C].rearrange("p (c e) -> p c e", e=C),
                        lo_tab,
                        idxsb[:, plan.idx_off[(s, 0)] : plan.idx_off[(s, 0)] + nclo * 8],
                        num_idxs=ni,
                        num_idxs_reg=ni,
                        elem_size=C,
                    )
                    add_dep_helper(gi.ins, cc.ins, True, "gather after AG")
                    g1 = None
                    if nchi:
                        g1 = gpool.tile([P, max_nch * C], BF16, tag="g1")
                        ni = nchi * P
                        gi = nc.gpsimd.dma_gather(
                            g1[:, : nchi * C].rearrange("p (c e) -> p c e", e=C),
                            hi_tab,
                            idxsb[
                                :,
                                plan.idx_off[(s, 1)] : plan.idx_off[(s, 1)] + nchi * 8,
                            ],
                            num_idxs=ni,
                            num_idxs_reg=ni,
                            elem_size=C,
                        )
                        add_dep_helper(gi.ins, cc.ins, True, "gather after AG")

                    nsr = nclo + nchi
                    pt = ppool.tile([P, max_nc_sr * P], BF16, tag="pt")
                    pc0 = plan.pm_off[s]
                    nc.sync.dma_start(
                        out=pt[:, : nsr * P], in_=PMAT[:, pc0 * P : (pc0 + nsr) * P]
                    )

                    for rl in range(4):
                        r = 4 * s + rl
                        chunks = []
                        lo_base = int(ncl[4 * s : r].sum())
                        for c in range(int(ncl[r])):
                            chunks.append((g0, lo_base + c, lo_base + c))
                        hi_base = int(nch[4 * s : r].sum())
                        for c in range(int(nch[r])):
                            chunks.append((g1, hi_base + c, nclo + hi_base + c))
                        ps = psc.tile([P, 512], F32, tag="sc")
                        nchunks = len(chunks)
                        for j in range(CK):
                            for k, (gt, gslot, pslot) in enumerate(chunks):
                                nc.tensor.matmul(
                                    ps[:, j * P : (j + 1) * P],
                                    lhsT=gt[
                                        :, gslot * C + j * P : gslot * C + j * P + P
                                    ],
                                    rhs=pt[:, pslot * P : (pslot + 1) * P],
                                    start=(k == 0),
                                    stop=(k == nchunks - 1),
                                )
                        for j in range(CK):
                            nc.scalar.activation(
                                hT[j][:, r * P : (r + 1) * P],
                                ps[:, j * P : (j + 1) * P],
                                mybir.ActivationFunctionType.Gelu,
                                bias=b2sb[:, l * CK + j : l * CK + j + 1],
                            )

            # ---------- readout ----------
            zsb = res.tile([1, NP], F32, name="z_sb", tag="z_sb")
            for nb in range(NB):
                ns = slice(nb * 512, (nb + 1) * 512)
                ps = pmm.tile([1, 512], F32, tag="mm")
                for i in range(CK):
                    nc.tensor.matmul(
                        ps[:],
                        lhsT=hwsb[:, i : i + 1],
                        rhs=hT[i][:, ns],
                        start=(i == 0),
                        stop=(i == CK - 1),
                    )
                nc.vector.tensor_copy(zsb[:, ns], ps[:])
            nc.gpsimd.dma_start(
                out=ZD[:].rearrange("(a n) -> a n", a=1), in_=zsb[:]
            )
            zcols = res.tile([P, T], BF16, name="zcols_sb", tag="zcols_sb")
            nc.gpsimd.dma_start(
                out=zcols[:], in_=ZD[:].rearrange("(t p) -> p t", p=P)
            )
            yp = pm2.tile([gpc, 1], F32, tag="m2")
            for t in range(T):
                nc.tensor.matmul(
                    yp[:],
                    lhsT=poolm[:, t * gpc : (t + 1) * gpc],
                    rhs=zcols[:, t : t + 1],
                    start=(t == 0),
                    stop=(t == T - 1),
                )
            ysb = res.tile([gpc, 1], F32, name="y_sb", tag="y_sb")
            nc.scalar.activation(
                ysb[:],
                yp[:],
                mybir.ActivationFunctionType.Identity,
                bias=hbsb[:],
                scale=rcsb[:],
            )
            nc.sync.dma_start(out=Y[:, :], in_=ysb[:])

    nc.compile()
    return nc


# ----------------------------------------------------------------------------
# Entry point
# ----------------------------------------------------------------------------

_CACHE = {}


def _run(inputs, n_cores=8, n_graphs=None, trace=False, trace_kwargs=None):
    plan, in_maps, _cuts = preprocess(inputs, n_cores=n_cores, n_graphs=n_graphs)
    key = (
        plan.NP,
        plan.IDXCOLS,
        plan.NCHUNKS,
        tuple(int(v) for v in plan.ncl),
        tuple(int(v) for v in plan.nch),
    )
    nc = _CACHE.get(key)
    if nc is None:
        nc = build_program(plan)
        _CACHE[key] = nc
    res = run_bass_kernel_spmd(
        nc, in_maps, list(range(n_cores)), trace=trace, **(trace_kwargs or {})
    )
    ys = [res.results[d]["y"] for d in range(n_cores)]
    out = np.concatenate(ys, axis=0).astype(np.float32)
    return out, res


def kernel(**inputs) -> np.ndarray:
    out, _ = _run(inputs, n_cores=8, n_graphs=256, trace=False)
    return out

